# revision 52
# baseline (speedup 1.0000x reference)
"""Trainium2 Bass kernel for nn_ExpertChoice (MoE routing + per-expert MLPs +
sum-weights router MLP + classification head), expert-parallel over 8 cores.

Self-contained: hardcodes full shapes (B=1024, N=E=8, D=768, K=4, NC=1000).

Per-core plan (core c == expert e):
  - router  : logits[b,:] = x[b,e,:] @ emb.T  (fp32 on PE), top-4 indices via
              DVE max8/max_index, gather token rows from x (bf16) with
              gpsimd dma_gather(transpose=True) -> selT feature-major.
  - sw MLP  : fc1 column-shard (this core's D cols of swW1), token-major out,
              PE-transpose, fc2 row-slice of swW2 -> partial logits [B,8];
              AllReduce (32KB, overlapped with expert fc1); softmax on chip.
  - experts : selT @ W1_e -> gelu -> hT (feature-major);  fc2 emits er
              token-major (lhsT = hT tiles), scaled by w[:,e] -> p.
  - combine : ReduceScatter(add) over p [B,KD] -> this core's 128-token slice
              of ws (bf16).
  - head    : PE-transpose ws slice, fc1 (full chW1) -> gelu -> hhT, fc2
              (full chW2) -> final logits for this core's tokens [128, 1000].
Host: concatenates the 8 token-slices -> [1024, 1000].
"""

import os
import numpy as np
import ml_dtypes

import concourse.bass as bass
import concourse.mybir as mybir
import concourse.tile as tile
from concourse import bacc
from concourse.masks import make_identity
from concourse.bass_utils import run_bass_kernel_spmd

F32 = mybir.dt.float32
BF16 = mybir.dt.bfloat16
I16 = mybir.dt.int16
U16 = mybir.dt.uint16
AF = mybir.ActivationFunctionType
ALU = mybir.AluOpType

NCORES = 8


class Cfg:
    def __init__(self, B=1024, D=768, NCLS=1000):
        self.B, self.D, self.NCLS = B, D, NCLS
        self.E = 8
        self.K = 4
        self.KD = self.K * D
        self.ND = self.E * D
        assert B % 128 == 0 and D % 128 == 0 and self.KD % 512 == 0
        self.DC = D // 128          # 128-chunks in D
        self.KC = self.KD // 128    # 128-chunks in KD
        self.NDC = self.ND // 128   # 128-chunks in ND
        self.TC = B // 128          # token chunks
        self.Bc = B // NCORES       # tokens per core after reduce-scatter
        self.NT = min(512, B)       # token free-dim tile for matmul
        self.NTC = B // self.NT
        self.GC = min(256, B)       # dma_gather chunk (hw limit < 512 idxs)
        self.NG = B // self.GC
        # sw fc1 col tiles (this core owns D cols), each <= 512
        self.SWT = [D // 2, D // 2] if D > 512 else [D]
        # head fc2 col tiles over NCLS, each <= 512
        nc_tiles = []
        rem = NCLS
        while rem > 0:
            t = min(500, rem)
            nc_tiles.append(t)
            rem -= t
        self.NCT = nc_tiles
        # W1/chW1 m-groups: groups of 4 chunks (512 cols)
        self.MG = self.KC // 4


def ceil_div(a, b):
    return (a + b - 1) // b


def build_nc(cfg: Cfg):
    c = cfg
    nc = bacc.Bacc("TRN2", target_bir_lowering=False, num_devices=NCORES)

    # ---- external I/O (per-core data differs only by in_map contents) ----
    xTe = nc.dram_tensor("xTe", [c.D, c.B], F32, kind="ExternalInput")
    embT = nc.dram_tensor("embT", [c.D, c.E], F32, kind="ExternalInput")
    xbf = nc.dram_tensor("xbf", [c.B * c.E, c.D], BF16, kind="ExternalInput")
    xfT = nc.dram_tensor("xfT", [c.ND, c.B], BF16, kind="ExternalInput")
    sw1s = nc.dram_tensor("sw1s", [c.ND, c.D], BF16, kind="ExternalInput")
    swb1c = nc.dram_tensor("swb1c", [128, c.DC], F32, kind="ExternalInput")
    sw2s = nc.dram_tensor("sw2s", [c.D, c.E], BF16, kind="ExternalInput")
    swb2 = nc.dram_tensor("swb2", [128, c.E], F32, kind="ExternalInput")
    w1e = nc.dram_tensor("w1e", [c.KD, c.KD], BF16, kind="ExternalInput")
    b1e = nc.dram_tensor("b1e", [c.KD], F32, kind="ExternalInput")
    w2e = nc.dram_tensor("w2e", [c.KD, c.KD], BF16, kind="ExternalInput")
    b2e = nc.dram_tensor("b2e", [128, c.KD], F32, kind="ExternalInput")
    chw1 = nc.dram_tensor("chw1", [c.KD, c.KD], BF16, kind="ExternalInput")
    chb1 = nc.dram_tensor("chb1", [c.KD], F32, kind="ExternalInput")
    chw2 = nc.dram_tensor("chw2", [c.KD, c.NCLS], BF16, kind="ExternalInput")
    chb2 = nc.dram_tensor("chb2", [128, c.NCLS], F32, kind="ExternalInput")
    onehot = nc.dram_tensor("onehot", [128, c.E], F32, kind="ExternalInput")
    out = nc.dram_tensor("out", [c.Bc, c.NCLS], F32, kind="ExternalOutput")

    rg = [list(range(NCORES))]

    with tile.TileContext(nc) as tc:
        # ------- DRAM scratch -------
        with tc.tile_pool(name="dram", bufs=1, space="DRAM") as dram:
            idx_dram = dram.tile([c.B, c.K], I16)
            wl_in = dram.tile([c.B, c.E], F32)
            wl_out = dram.tile([c.B, c.E], F32, addr_space="Shared")
            p_dram = dram.tile([c.B, c.KD], BF16)
            ws_dram = dram.tile([c.B * 3 // 4 // NCORES, c.KD], BF16)
            ws_dram2 = dram.tile([c.B // 4 // NCORES, c.KD], BF16)

            _build_body(nc, tc, c, rg, locals())
    nc.finalize()
    return nc


def _dbg_out(nc, tc, c, out, src_ap):
    # debug epilogue: route some live data to `out` so truncated builds run
    import contextlib
    with tc.tile_pool(name="dbgo", bufs=1) as dp:
        dbg = dp.tile([c.Bc, c.NCLS], F32)
        nc.vector.memset(dbg, 0.0)
        s0, s1 = min(c.Bc, src_ap.shape[0]), min(c.NCLS, src_ap.shape[1])
        nc.vector.tensor_copy(dbg[0:s0, 0:s1], src_ap[0:s0, 0:s1])
        nc.sync.dma_start(out[:, :], dbg)


def _build_body(nc, tc, c, rg, T):
    xTe, embT, xbf, xfT = T["xTe"], T["embT"], T["xbf"], T["xfT"]
    sw1s, swb1c, sw2s, swb2 = T["sw1s"], T["swb1c"], T["sw2s"], T["swb2"]
    w1e, b1e, w2e, b2e = T["w1e"], T["b1e"], T["w2e"], T["b2e"]
    chw1, chb1, chw2, chb2 = T["chw1"], T["chb1"], T["chw2"], T["chb2"]
    onehot, out = T["onehot"], T["out"]
    idx_dram, wl_in, wl_out = T["idx_dram"], T["wl_in"], T["wl_out"]
    p_dram, ws_dram = T["p_dram"], T["ws_dram"]
    ws_dram2 = T["ws_dram2"]

    import contextlib

    phase = int(os.environ.get("KPHASE", "9"))
    ctx = contextlib.ExitStack()
    with ctx:
        const = ctx.enter_context(tc.tile_pool(name="const", bufs=1))
        ident = const.tile([128, 128], BF16)
        make_identity(nc, ident)

        # const tiles allocated here; DMA issue deferred until after the
        # router input loads so HWDGE serves the critical path first.
        swb1_sb = const.tile([128, c.DC], F32)
        swb2_sb = const.tile([128, c.E], F32)
        chb2_sb = const.tile([128, c.NCLS], F32)
        oh_sb = const.tile([128, c.E], F32)
        b1_sb = const.tile([128, c.KC], F32)
        chb1_sb = const.tile([128, c.KC], F32)

        def load_consts():
            nc.sync.dma_start(swb1_sb, swb1c[:, :])
            nc.sync.dma_start(swb2_sb, swb2[:, :])
            nc.sync.dma_start(chb2_sb, chb2[:, :])
            nc.sync.dma_start(oh_sb, onehot[:, :])
            nc.sync.dma_start(b1_sb, b1e.rearrange("(k p) -> p k", p=128))
            nc.sync.dma_start(chb1_sb, chb1.rearrange("(k p) -> p k", p=128))

        # Long-lived tiles, allocated in reverse order of release so pool
        # frees stay LIFO (the Tile allocator requires stack order).
        out_sb = const.tile([c.Bc, c.NCLS], F32)
        wcol_pool = ctx.enter_context(tc.tile_pool(name="wcol", bufs=1))
        w_col = wcol_pool.tile([128, c.TC], F32)
        wsT = ctx.enter_context(tc.tile_pool(name="wsT", bufs=1)).tile(
            [128, c.KC, c.Bc], BF16
        )
        hhT = ctx.enter_context(tc.tile_pool(name="hhT", bufs=1)).tile(
            [128, c.KC, c.Bc], BF16
        )
        hT = ctx.enter_context(tc.tile_pool(name="hT", bufs=1)).tile(
            [128, c.KC, c.B], BF16
        )

        if phase <= -1:
            _dbg_out(nc, tc, c, out, b1_sb)
            return
        # ---------------- Phase 1: router (fp32) + top-k + gather ----------
        # token-chunk-major layout so each (j, nt) gather dst is contiguous
        sel_es = contextlib.ExitStack()
        selT = sel_es.enter_context(tc.tile_pool(name="selT", bufs=1)).tile(
            [128, c.NG, c.K * c.DC, c.GC], BF16
        )
        # first expert-fc1 weight m-block, preloaded as two half-K tiles
        # during the sum-weights phase (the full-block pool opens at fc1).
        w1v = w1e.rearrange(
            "(kh kc p) (mg m) -> mg kh p kc m", p=128, kh=2, m=512
        )
        w1a_es = contextlib.ExitStack()
        w1a_pool = w1a_es.enter_context(tc.tile_pool(name="w1a", bufs=1))
        w1a = [w1a_pool.tile([128, c.KC // 2, 512], BF16, name=f"w1a{kh}")
               for kh in range(2)]

        # ---------------- Phase 2 pools (sum-weights fc1, feature-major) ---
        # swhT[f, b] = gelu(sum_k swW1[k, e*D+f] * xf[b, k]): lhsT = sw1s
        # chunk [128k, 128m], rhs = xfT chunk [128k, 512 tokens].
        swhT_es = contextlib.ExitStack()
        swhT = swhT_es.enter_context(tc.tile_pool(name="swhT", bufs=1)).tile(
            [128, c.DC, c.B], BF16
        )
        KB = 8                     # kc chunks per streamed block
        NKB = c.NDC // KB
        sw1v = sw1s.rearrange("(kb k p) m -> kb p k m", p=128, kb=NKB)
        xfv = xfT.rearrange("(kb k p) b -> kb p k b", p=128, kb=NKB)
        from concourse import library_config
        S = c.B // 16

        with tc.tile_pool(name="sww", bufs=2) as sww_pool, \
             tc.tile_pool(name="swx", bufs=2) as swx_pool, \
             tc.tile_pool(name="swf1_psum", bufs=1, space="PSUM") as swf1_psum:
            # first sw-fc1 block loads go ahead of the router's x load so the
            # PE can move straight from router matmuls into sw fc1.
            wblk00 = sww_pool.tile([128, KB, c.D], BF16, tag="sww", name="sww")
            xblk00 = swx_pool.tile([128, KB, 512], BF16, tag="swx", name="swx")

            with tc.tile_pool(name="rt", bufs=1) as rt_pool, \
                 tc.tile_pool(name="rt_psum", bufs=1, space="PSUM") as rt_psum:
                embT_sb = rt_pool.tile([128, c.DC, c.E], F32)
                nc.sync.dma_start(
                    embT_sb, embT.rearrange("(kc p) e -> p kc e", p=128)
                )
                xTv = xTe.rearrange("(kh kc p) b -> kh p kc b", p=128, kh=2)
                xTh0 = rt_pool.tile([128, 3, c.B], F32, tag="xTh", name="xTh")
                nc.sync.dma_start(xTh0, xTv[0])
                nc.sync.dma_start(wblk00, sw1v[0])
                nc.sync.dma_start(xblk00, xfv[0, :, :, 0:512])

                rt_ps = rt_psum.tile([128, c.TC, c.E], F32)
                for kh in range(2):
                    if kh == 0:
                        xTh = xTh0
                    else:
                        xTh = rt_pool.tile([128, 3, c.B], F32, tag="xTh",
                                           name="xTh")
                        nc.sync.dma_start(xTh, xTv[1])
                    for kc in range(3):
                        for t in range(c.TC):
                            nc.tensor.matmul(
                                rt_ps[:, t, :].opt(),
                                lhsT=xTh[:, kc, t * 128:(t + 1) * 128].opt(),
                                rhs=embT_sb[:, kh * 3 + kc, :].opt(),
                                start=(kh == 0 and kc == 0),
                                stop=(kh == 1 and kc == 2),
                            )
                idx_sb = rt_pool.tile([128, c.TC, c.K], I16, tag="idxsb",
                                      bufs=1)
                for t in range(c.TC):
                    lg = rt_pool.tile([128, c.E], F32, tag="lg", bufs=2)
                    nc.scalar.activation(lg, rt_ps[:, t, :].opt(), AF.Copy)
                    vals = rt_pool.tile([128, 8], F32, tag="vals", bufs=2)
                    nc.vector.max(out=vals, in_=lg)
                    idx8 = rt_pool.tile([128, 8], U16, tag="idx8", bufs=2)
                    nc.vector.max_index(out=idx8, in_max=vals, in_values=lg)
                    iota = rt_pool.tile([128, c.K], I16, tag="iota", bufs=2)
                    nc.gpsimd.iota(
                        iota, pattern=[[0, c.K]], base=t * 128 * c.E,
                        channel_multiplier=c.E,
                    )
                    nc.vector.tensor_add(
                        idx_sb[:, t, :].opt(), iota, idx8[:, 0:c.K].bitcast(I16)
                    )
                nc.sync.dma_start(
                    idx_dram.rearrange("(t p) j -> p t j", p=128), idx_sb
                )

                if phase <= 0:
                    _dbg_out(nc, tc, c, out, idx_sb[:, 0, :])
                    return

            for tb in range(2):
                psums = [
                    swf1_psum.tile([128, 512], F32, name=f"swps{mc}")
                    for mc in range(c.DC)
                ]
                for kb in range(NKB):
                    if tb == 0 and kb == 0:
                        wblk, xblk = wblk00, xblk00
                    else:
                        wblk = sww_pool.tile([128, KB, c.D], BF16, tag="sww",
                                             name="sww")
                        nc.sync.dma_start(wblk, sw1v[kb])
                        xblk = swx_pool.tile([128, KB, 512], BF16, tag="swx",
                                             name="swx")
                        nc.sync.dma_start(
                            xblk, xfv[kb, :, :, tb * 512:(tb + 1) * 512]
                        )
                    for mc in range(c.DC):
                        for k in range(KB):
                            nc.tensor.matmul(
                                psums[mc],
                                lhsT=wblk[:, k,
                                          mc * 128:(mc + 1) * 128].opt(),
                                rhs=xblk[:, k, :].opt(),
                                start=(kb == 0 and k == 0),
                                stop=(kb == NKB - 1 and k == KB - 1),
                            )
                for mc in range(c.DC):
                    nc.scalar.activation(
                        swhT[:, mc, tb * 512:(tb + 1) * 512].opt(),
                        psums[mc], AF.Gelu, bias=swb1_sb[:, mc:mc + 1],
                    )
                if tb == 0:
                    # ---- gather (indices -> selT) + consts, issued here so
                    # their DMAs interleave with the tb=1 stream ----------
                    nc.gpsimd.load_library(library_config.mlp)
                    with tc.tile_pool(name="gat", bufs=1) as gat_pool:
                        idxw_js = gat_pool.tile([128, S, c.K], I16)
                        iv = idx_dram.rearrange("(s p) j -> p s j", p=16)
                        for r in range(8):
                            nc.sync.dma_start(
                                idxw_js[16 * r:16 * r + 16, :, :], iv
                            )
                        idxw = gat_pool.tile([128, c.K, S], I16)
                        nc.vector.tensor_copy(
                            idxw, idxw_js[:, :, :].rearrange("p s j -> p j s")
                        )
                        load_consts()
                        for kh in range(2):
                            nc.sync.dma_start(w1a[kh], w1v[0, kh])
                        for g in range(c.NG):
                            for j in range(c.K):
                                nc.gpsimd.dma_gather(
                                    out_ap=selT[:, g, c.DC * j:c.DC * (j + 1), :],
                                    in_ap=xbf[:, :],
                                    idxs_ap=idxw[:, j, g * c.GC // 16:
                                                 (g + 1) * c.GC // 16],
                                    num_idxs=c.GC,
                                    num_idxs_reg=c.GC,
                                    elem_size=c.D,
                                    transpose=True,
                                )

        if phase <= 1:
            _dbg_out(nc, tc, c, out, selT[:, 0, 0, :])
            return

        with tc.tile_pool(name="swf2_psum", bufs=2, space="PSUM") as swf2_psum, \
             tc.tile_pool(name="swmisc", bufs=3) as swmisc:
            # fc2: partial logits, token-major [B, E]
            sw2_sb = swmisc.tile([128, c.DC, c.E], BF16, bufs=1)
            nc.sync.dma_start(sw2_sb, sw2s.rearrange("(kc p) e -> p kc e", p=128))
            wl_sb = swmisc.tile([128, c.TC, c.E], F32, bufs=1)
            for t in range(c.TC):
                ps = swf2_psum.tile([128, c.E], F32, name="swf2p")
                for kc in range(c.DC):
                    nc.tensor.matmul(
                        ps,
                        lhsT=swhT[:, kc, t * 128:(t + 1) * 128].opt(),
                        rhs=sw2_sb[:, kc, :].opt(),
                        start=(kc == 0),
                        stop=(kc == c.DC - 1),
                    )
                nc.scalar.activation(wl_sb[:, t, :].opt(), ps, AF.Copy)
            nc.sync.dma_start(
                wl_in.rearrange("(t p) e -> p t e", p=128), wl_sb
            )

        # AllReduce the sum-weights partial logits (tiny, overlaps fc1)
        nc.gpsimd.collective_compute(
            "AllReduce", ALU.add, replica_groups=rg,
            ins=[wl_in.opt()], outs=[wl_out.opt()],
        )
        swhT_es.close()

        if phase <= 2:
            _dbg_out(nc, tc, c, out, hT[:, 0, :] if False else selT[:, 0, 0, :])
            return
        # ---------------- Phase 3: expert fc1 ------------------------------
        KHC = c.KC // 2
        with tc.tile_pool(name="w1blk", bufs=2) as w1_pool, \
             tc.tile_pool(name="f1_psum", bufs=4, space="PSUM") as f1_psum:
            for mg in range(c.MG):
                if mg == 0:
                    def lhs_at(kc, mc):
                        return w1a[kc // KHC][:, kc % KHC,
                                              mc * 128:(mc + 1) * 128]
                else:
                    blk = w1_pool.tile([128, c.KC, 512], BF16, tag="w1b")
                    nc.sync.dma_start(
                        blk,
                        w1e[:, mg * 512:(mg + 1) * 512].rearrange(
                            "(kc p) m -> p kc m", p=128
                        ),
                    )

                    def lhs_at(kc, mc, blk=blk):
                        return blk[:, kc, mc * 128:(mc + 1) * 128]
                for mc in range(4):
                    m = mg * 4 + mc
                    for g in range(c.NG):
                        ps = f1_psum.tile([128, c.GC], F32, name="f1ps")
                        for kc in range(c.KC):
                            nc.tensor.matmul(
                                ps,
                                lhsT=lhs_at(kc, mc).opt(),
                                rhs=selT[:, g, kc, :].opt(),
                                start=(kc == 0),
                                stop=(kc == c.KC - 1),
                            )
                        nc.scalar.activation(
                            hT[:, m, g * c.GC:(g + 1) * c.GC].opt(),
                            ps, AF.Gelu, bias=b1_sb[:, m:m + 1],
                        )
        w1a_es.close()
        sel_es.close()

        if phase <= 3:
            _dbg_out(nc, tc, c, out, hT[:, 0, :])
            return
        # ---------------- softmax of routing weights (after AllReduce) -----
        # w_col[t] = softmax(wl + swb2)[:, e]  as per-partition scalars
        with tc.tile_pool(name="smx", bufs=2) as smx:
            wlf_all = smx.tile([128, c.TC, c.E], F32, bufs=1)
            nc.sync.dma_start(
                wlf_all, wl_out.rearrange("(t p) e -> p t e", p=128)
            )
            for t in range(c.TC):
                wlf = smx.tile([128, c.E], F32, tag="wlf")
                nc.vector.tensor_add(wlf, wlf_all[:, t, :].opt(), swb2_sb)
                mx = smx.tile([128, 1], F32, tag="mx")
                nc.vector.reduce_max(out=mx, in_=wlf, axis=mybir.AxisListType.X)
                nmx = smx.tile([128, 1], F32, tag="nmx")
                nc.vector.tensor_scalar_mul(nmx, mx, -1.0)
                ex = smx.tile([128, c.E], F32, tag="ex")
                sm = smx.tile([128, 1], F32, tag="sm")
                nc.scalar.activation(ex, wlf, AF.Exp, bias=nmx, accum_out=sm)
                rs = smx.tile([128, 1], F32, tag="rs")
                nc.vector.reciprocal(rs, sm)
                # pick this expert's column via onehot + row-sum, then scale
                sel = smx.tile([128, c.E], F32, tag="sel")
                nc.vector.tensor_mul(sel, ex, oh_sb)
                num = smx.tile([128, 1], F32, tag="num")
                nc.vector.reduce_sum(out=num, in_=sel, axis=mybir.AxisListType.X)
                nc.vector.tensor_tensor(
                    out=w_col[:, t:t + 1], in0=num, in1=rs, op=ALU.mult
                )

        # ---------------- Phase 4: expert fc2 (token-major) + scale -------
        # Token chunks split 768/256 so the final ReduceScatter is small and
        # its latency hides under the part-a head compute.
        NCOL = c.KD // 512
        TA = 6                  # token chunks in part a (768 tokens)
        TB = c.TC - TA          # part b (256 tokens)
        BA = TA * 128
        RA, RB = BA // NCORES, (c.B - BA) // NCORES   # 96 / 32 rows per core
        # w2 streamed as half-K blocks [128, 12, 512] so SBUF fits alongside
        # the chw1 (head fc1) prefetch pool.
        w2v = w2e.rearrange(
            "(kh kc p) (ncol m) -> ncol kh p kc m", p=128, m=512, kh=2
        )
        chv = chw1.rearrange("(kc p) (mg m) -> mg p kc m", p=128, m=512)
        pva = p_dram.rearrange(
            "(q ti p) (ncol m) -> q ncol p ti m", p=128, q=c.TC // 2, m=512
        )
        ch1_pool = ctx.enter_context(tc.tile_pool(name="ch1blk", bufs=3))
        ch1t = [ch1_pool.tile([128, c.KC, 512], BF16, tag="c1b", name="c1b")
                for mg in range(c.MG)]
        KH = c.KC // 2
        with tc.tile_pool(name="w2p", bufs=3) as w2_pool, \
             tc.tile_pool(name="f2_psum", bufs=2, space="PSUM") as f2_psum, \
             tc.tile_pool(name="pout", bufs=3) as p_pool:
            b2_sb = p_pool.tile([128, c.KD], F32, bufs=1)
            it = 0
            # ncol-outer within each token part: each w2 column block is
            # loaded once per part (2x total) instead of once per q group.
            for part, qr in ((0, range(0, TA // 2)), (1, range(TA // 2, 4))):
                for ncol in range(NCOL):
                    blks = []
                    for kh in range(2):
                        b_ = w2_pool.tile([128, KH, 512], BF16, tag="w2b",
                                          name="w2b")
                        nc.sync.dma_start(b_, w2v[ncol, kh])
                        blks.append(b_)
                    if it == 0:
                        nc.sync.dma_start(b2_sb, b2e[:, :])
                    if it < 3:              # prefetch first head fc1 blocks
                        nc.sync.dma_start(ch1t[it], chv[it])
                    it += 1
                    for q in qr:
                        psums = [
                            f2_psum.tile([128, 512], F32, name=f"f2ps{ti}")
                            for ti in range(2)
                        ]
                        pbig = p_pool.tile([128, 2, 512], BF16, tag="pbig")
                        for kc in range(c.KC):
                            for ti in range(2):
                                t = q * 2 + ti
                                nc.tensor.matmul(
                                    psums[ti],
                                    lhsT=hT[:, kc,
                                            t * 128:(t + 1) * 128].opt(),
                                    rhs=blks[kc // KH][:, kc % KH, :].opt(),
                                    start=(kc == 0),
                                    stop=(kc == c.KC - 1),
                                )
                        for ti in range(2):
                            t = q * 2 + ti
                            er = p_pool.tile([128, 512], F32, tag="er")
                            nc.vector.tensor_add(
                                er, psums[ti],
                                b2_sb[:, ncol * 512:(ncol + 1) * 512],
                            )
                            nc.vector.tensor_scalar_mul(
                                pbig[:, ti, :].opt(), er, w_col[:, t:t + 1]
                            )
                        nc.sync.dma_start(pva[q, ncol], pbig)
                if part == 0:
                    nc.gpsimd.collective_compute(
                        "ReduceScatter", ALU.add, replica_groups=rg,
                        ins=[p_dram[0:BA, :].opt()], outs=[ws_dram.opt()],
                    )
            nc.gpsimd.collective_compute(
                "ReduceScatter", ALU.add, replica_groups=rg,
                ins=[p_dram[BA:c.B, :].opt()], outs=[ws_dram2.opt()],
            )

        if phase <= 4:
            _dbg_out(nc, tc, c, out, w_col)
            return

        # ---------------- Phase 6: head on this core's Bc tokens -----------
        ch2_es = contextlib.ExitStack()
        ch2_pool = ch2_es.enter_context(tc.tile_pool(name="ch2", bufs=1))
        ch2t = []
        col0 = 0
        for i, ncols in enumerate(c.NCT):
            blk2 = ch2_pool.tile([128, c.KC, ncols], BF16, name=f"c2b{i}")
            nc.sync.dma_start(
                blk2,
                chw2[:, col0:col0 + ncols].rearrange("(kc p) m -> p kc m", p=128),
            )
            ch2t.append(blk2)
            col0 += ncols

        with tc.tile_pool(name="wst", bufs=2) as wst_pool, \
             tc.tile_pool(name="wst_psum", bufs=2, space="PSUM") as wst_psum, \
             tc.tile_pool(name="h1_psum", bufs=2, space="PSUM") as h1_psum:
            # part a (RA tokens, RS1 done early) runs while the small RS2 is
            # in flight; strict (a_i, b_i) pairs keep each chw1 block's two
            # uses adjacent so the 3-buffer stream window suffices.
            wsa = wst_pool.tile([RA, c.KD], BF16, bufs=1)
            nc.sync.dma_start(wsa, ws_dram[:, :])
            wsb = wst_pool.tile([RB, c.KD], BF16, bufs=1)
            nc.sync.dma_start(wsb, ws_dram2[:, :])

            def trans_part(src, r0, rn):
                for kc in range(c.KC):
                    tp = wst_psum.tile([128, rn], BF16, name="wstp")
                    nc.tensor.transpose(
                        tp, src[:, kc * 128:(kc + 1) * 128],
                        ident[0:rn, 0:rn],
                    )
                    nc.scalar.activation(
                        wsT[:, kc, r0:r0 + rn].opt(), tp, AF.Copy
                    )

            def fc1_part(mg, r0, rn):
                for mc in range(4):
                    m = mg * 4 + mc
                    ps = h1_psum.tile([128, rn], F32, name="h1ps")
                    for kc in range(c.KC):
                        nc.tensor.matmul(
                            ps,
                            lhsT=ch1t[mg][:, kc,
                                          mc * 128:(mc + 1) * 128].opt(),
                            rhs=wsT[:, kc, r0:r0 + rn].opt(),
                            start=(kc == 0),
                            stop=(kc == c.KC - 1),
                        )
                    nc.scalar.activation(
                        hhT[:, m, r0:r0 + rn].opt(), ps, AF.Gelu,
                        bias=chb1_sb[:, m:m + 1],
                    )

            trans_part(wsa, 0, RA)
            fc1_part(0, 0, RA)
            trans_part(wsb, RA, RB)
            for mg in range(c.MG):
                if mg + 3 < c.MG:
                    nc.sync.dma_start(ch1t[mg + 3], chv[mg + 3])
                if mg > 0:
                    fc1_part(mg, 0, RA)
                fc1_part(mg, RA, RB)
        with tc.tile_pool(name="h2_psum", bufs=2, space="PSUM") as h2_psum:
            col0 = 0
            for i, ncols in enumerate(c.NCT):
                ps = h2_psum.tile([c.Bc, ncols], F32, name="h2ps")
                for kc in range(c.KC):
                    nc.tensor.matmul(
                        ps,
                        lhsT=hhT[:, kc, :].opt(),
                        rhs=ch2t[i][:, kc, :].opt(),
                        start=(kc == 0),
                        stop=(kc == c.KC - 1),
                    )
                nc.vector.tensor_add(
                    out_sb[:, col0:col0 + ncols], ps,
                    chb2_sb[0:c.Bc, col0:col0 + ncols],
                )
                col0 += ncols
            nc.sync.dma_start(out[:, :], out_sb)
        ch2_es.close()


# ======================= host side =======================================

def prep_in_maps(inputs, cfg: Cfg):
    c = cfg
    bf = ml_dtypes.bfloat16
    x = np.asarray(inputs["x"], np.float32)
    emb = np.asarray(inputs["emb"], np.float32)
    W1 = np.asarray(inputs["W1"], np.float32)
    b1 = np.asarray(inputs["b1"], np.float32)
    W2 = np.asarray(inputs["W2"], np.float32)
    b2 = np.asarray(inputs["b2"], np.float32)
    swW1 = np.asarray(inputs["swW1"], np.float32)
    swb1 = np.asarray(inputs["swb1"], np.float32)
    swW2 = np.asarray(inputs["swW2"], np.float32)
    swb2 = np.asarray(inputs["swb2"], np.float32)
    chW1 = np.asarray(inputs["chW1"], np.float32)
    chb1 = np.asarray(inputs["chb1"], np.float32)
    chW2 = np.asarray(inputs["chW2"], np.float32)
    chb2 = np.asarray(inputs["chb2"], np.float32)

    embT = np.ascontiguousarray(emb.T)
    xbf = np.ascontiguousarray(x.reshape(c.B * c.E, c.D)).astype(bf)
    xfT = np.ascontiguousarray(x.reshape(c.B, c.ND).T).astype(bf)
    chw1_b = chW1.astype(bf)
    chw2_b = chW2.astype(bf)
    swb2_r = np.ascontiguousarray(np.broadcast_to(swb2.reshape(1, c.E), (128, c.E)))
    chb2_r = np.ascontiguousarray(np.broadcast_to(chb2.reshape(1, c.NCLS), (128, c.NCLS)))

    in_maps = []
    for e in range(NCORES):
        oh = np.zeros((128, c.E), np.float32)
        oh[:, e] = 1.0
        m = {
            "xTe": np.ascontiguousarray(x[:, e, :].T),
            "embT": embT,
            "xbf": xbf,
            "xfT": xfT,
            "sw1s": np.ascontiguousarray(swW1[:, e * c.D:(e + 1) * c.D]).astype(bf),
            "swb1c": np.ascontiguousarray(
                swb1[e * c.D:(e + 1) * c.D].reshape(c.DC, 128).T
            ),
            "sw2s": np.ascontiguousarray(swW2[e * c.D:(e + 1) * c.D, :]).astype(bf),
            "swb2": swb2_r,
            "w1e": W1[e].astype(bf),
            "b1e": b1[e],
            "w2e": W2[e].astype(bf),
            "b2e": np.ascontiguousarray(np.broadcast_to(b2[e].reshape(1, c.KD), (128, c.KD))),
            "chw1": chw1_b,
            "chb1": chb1,
            "chw2": chw2_b,
            "chb2": chb2_r,
            "onehot": oh,
        }
        in_maps.append(m)
    return in_maps


_CACHE = {}


def kernel(**inputs) -> np.ndarray:
    cfg = Cfg()
    key = "nc" + os.environ.get("KPHASE", "9")
    if key not in _CACHE:
        _CACHE[key] = build_nc(cfg)
    nc = _CACHE[key]
    in_maps = prep_in_maps(inputs, cfg)
    res = run_bass_kernel_spmd(
        nc, in_maps, core_ids=list(range(NCORES)),
        trace=bool(int(os.environ.get("KBENCH_TRACE", "0"))),
    )
    _CACHE["last_results"] = res
    ra, rb = cfg.B * 3 // 4 // NCORES, cfg.B // 4 // NCORES
    ba = cfg.B * 3 // 4
    outp = np.empty((cfg.B, cfg.NCLS), np.float32)
    for cix in range(NCORES):
        o = res.results[cix]["out"]
        outp[cix * ra:(cix + 1) * ra] = o[0:ra]
        outp[ba + cix * rb:ba + (cix + 1) * rb] = o[ra:]
    return outp



# revision 53
# speedup vs baseline: 1.0126x; 1.0126x over previous
"""Trainium2 Bass kernel for nn_ExpertChoice (MoE routing + per-expert MLPs +
sum-weights router MLP + classification head), expert-parallel over 8 cores.

Self-contained: hardcodes full shapes (B=1024, N=E=8, D=768, K=4, NC=1000).

Per-core plan (core c == expert e):
  - router  : logits[b,:] = x[b,e,:] @ emb.T  (fp32 on PE), top-4 indices via
              DVE max8/max_index, gather token rows from x (bf16) with
              gpsimd dma_gather(transpose=True) -> selT feature-major.
  - sw MLP  : fc1 column-shard (this core's D cols of swW1), token-major out,
              PE-transpose, fc2 row-slice of swW2 -> partial logits [B,8];
              AllReduce (32KB, overlapped with expert fc1); softmax on chip.
  - experts : selT @ W1_e -> gelu -> hT (feature-major);  fc2 emits er
              token-major (lhsT = hT tiles), scaled by w[:,e] -> p.
  - combine : ReduceScatter(add) over p [B,KD] -> this core's 128-token slice
              of ws (bf16).
  - head    : PE-transpose ws slice, fc1 (full chW1) -> gelu -> hhT, fc2
              (full chW2) -> final logits for this core's tokens [128, 1000].
Host: concatenates the 8 token-slices -> [1024, 1000].
"""

import os
import numpy as np
import ml_dtypes

import concourse.bass as bass
import concourse.mybir as mybir
import concourse.tile as tile
from concourse import bacc
from concourse.masks import make_identity
from concourse.bass_utils import run_bass_kernel_spmd

F32 = mybir.dt.float32
BF16 = mybir.dt.bfloat16
I16 = mybir.dt.int16
U16 = mybir.dt.uint16
AF = mybir.ActivationFunctionType
ALU = mybir.AluOpType

NCORES = 8


class Cfg:
    def __init__(self, B=1024, D=768, NCLS=1000):
        self.B, self.D, self.NCLS = B, D, NCLS
        self.E = 8
        self.K = 4
        self.KD = self.K * D
        self.ND = self.E * D
        assert B % 128 == 0 and D % 128 == 0 and self.KD % 512 == 0
        self.DC = D // 128          # 128-chunks in D
        self.KC = self.KD // 128    # 128-chunks in KD
        self.NDC = self.ND // 128   # 128-chunks in ND
        self.TC = B // 128          # token chunks
        self.Bc = B // NCORES       # tokens per core after reduce-scatter
        self.NT = min(512, B)       # token free-dim tile for matmul
        self.NTC = B // self.NT
        self.GC = min(256, B)       # dma_gather chunk (hw limit < 512 idxs)
        self.NG = B // self.GC
        # sw fc1 col tiles (this core owns D cols), each <= 512
        self.SWT = [D // 2, D // 2] if D > 512 else [D]
        # head fc2 col tiles over NCLS, each <= 512
        nc_tiles = []
        rem = NCLS
        while rem > 0:
            t = min(500, rem)
            nc_tiles.append(t)
            rem -= t
        self.NCT = nc_tiles
        # W1/chW1 m-groups: groups of 4 chunks (512 cols)
        self.MG = self.KC // 4


def ceil_div(a, b):
    return (a + b - 1) // b


def build_nc(cfg: Cfg):
    c = cfg
    nc = bacc.Bacc("TRN2", target_bir_lowering=False, num_devices=NCORES)

    # ---- external I/O (per-core data differs only by in_map contents) ----
    xTe = nc.dram_tensor("xTe", [c.D, c.B], F32, kind="ExternalInput")
    embT = nc.dram_tensor("embT", [c.D, c.E], F32, kind="ExternalInput")
    xbf = nc.dram_tensor("xbf", [c.B * c.E, c.D], BF16, kind="ExternalInput")
    xfT = nc.dram_tensor("xfT", [c.ND, c.B], BF16, kind="ExternalInput")
    sw1s = nc.dram_tensor("sw1s", [c.ND, c.D], BF16, kind="ExternalInput")
    swb1c = nc.dram_tensor("swb1c", [128, c.DC], F32, kind="ExternalInput")
    sw2s = nc.dram_tensor("sw2s", [c.D, c.E], BF16, kind="ExternalInput")
    swb2 = nc.dram_tensor("swb2", [128, c.E], F32, kind="ExternalInput")
    w1e = nc.dram_tensor("w1e", [c.KD, c.KD], BF16, kind="ExternalInput")
    b1e = nc.dram_tensor("b1e", [c.KD], F32, kind="ExternalInput")
    w2e = nc.dram_tensor("w2e", [c.KD, c.KD], BF16, kind="ExternalInput")
    b2e = nc.dram_tensor("b2e", [128, c.KD], F32, kind="ExternalInput")
    chw1 = nc.dram_tensor("chw1", [c.KD, c.KD], BF16, kind="ExternalInput")
    chb1 = nc.dram_tensor("chb1", [c.KD], F32, kind="ExternalInput")
    chw2 = nc.dram_tensor("chw2", [c.KD, c.NCLS], BF16, kind="ExternalInput")
    chb2 = nc.dram_tensor("chb2", [128, c.NCLS], F32, kind="ExternalInput")
    onehot = nc.dram_tensor("onehot", [128, c.E], F32, kind="ExternalInput")
    out = nc.dram_tensor("out", [c.Bc, c.NCLS], F32, kind="ExternalOutput")

    rg = [list(range(NCORES))]

    with tile.TileContext(nc) as tc:
        # ------- DRAM scratch -------
        with tc.tile_pool(name="dram", bufs=1, space="DRAM") as dram:
            idx_dram = dram.tile([c.B, c.K], I16)
            wl_in = dram.tile([c.B, c.E], F32)
            wl_out = dram.tile([c.B, c.E], F32, addr_space="Shared")
            p_dram = dram.tile([c.B, c.KD], BF16)
            ws_dram = dram.tile([c.B * 3 // 4 // NCORES, c.KD], BF16)
            ws_dram2 = dram.tile([c.B // 4 // NCORES, c.KD], BF16)

            _build_body(nc, tc, c, rg, locals())
    nc.finalize()
    return nc


def _dbg_out(nc, tc, c, out, src_ap):
    # debug epilogue: route some live data to `out` so truncated builds run
    import contextlib
    with tc.tile_pool(name="dbgo", bufs=1) as dp:
        dbg = dp.tile([c.Bc, c.NCLS], F32)
        nc.vector.memset(dbg, 0.0)
        s0, s1 = min(c.Bc, src_ap.shape[0]), min(c.NCLS, src_ap.shape[1])
        nc.vector.tensor_copy(dbg[0:s0, 0:s1], src_ap[0:s0, 0:s1])
        nc.sync.dma_start(out[:, :], dbg)


def _build_body(nc, tc, c, rg, T):
    xTe, embT, xbf, xfT = T["xTe"], T["embT"], T["xbf"], T["xfT"]
    sw1s, swb1c, sw2s, swb2 = T["sw1s"], T["swb1c"], T["sw2s"], T["swb2"]
    w1e, b1e, w2e, b2e = T["w1e"], T["b1e"], T["w2e"], T["b2e"]
    chw1, chb1, chw2, chb2 = T["chw1"], T["chb1"], T["chw2"], T["chb2"]
    onehot, out = T["onehot"], T["out"]
    idx_dram, wl_in, wl_out = T["idx_dram"], T["wl_in"], T["wl_out"]
    p_dram, ws_dram = T["p_dram"], T["ws_dram"]
    ws_dram2 = T["ws_dram2"]

    import contextlib

    phase = int(os.environ.get("KPHASE", "9"))
    ctx = contextlib.ExitStack()
    with ctx:
        const = ctx.enter_context(tc.tile_pool(name="const", bufs=1))
        ident = const.tile([128, 128], BF16)
        make_identity(nc, ident)

        # const tiles allocated here; DMA issue deferred until after the
        # router input loads so HWDGE serves the critical path first.
        swb1_sb = const.tile([128, c.DC], F32)
        swb2_sb = const.tile([128, c.E], F32)
        chb2_sb = const.tile([128, c.NCLS], F32)
        oh_sb = const.tile([128, c.E], F32)
        b1_sb = const.tile([128, c.KC], F32)
        chb1_sb = const.tile([128, c.KC], F32)

        def load_consts():
            nc.sync.dma_start(swb1_sb, swb1c[:, :])
            nc.sync.dma_start(swb2_sb, swb2[:, :])
            nc.sync.dma_start(chb2_sb, chb2[:, :])
            nc.sync.dma_start(oh_sb, onehot[:, :])
            nc.sync.dma_start(b1_sb, b1e.rearrange("(k p) -> p k", p=128))
            nc.sync.dma_start(chb1_sb, chb1.rearrange("(k p) -> p k", p=128))

        # Long-lived tiles, allocated in reverse order of release so pool
        # frees stay LIFO (the Tile allocator requires stack order).
        out_sb = const.tile([c.Bc, c.NCLS], F32)
        wcol_pool = ctx.enter_context(tc.tile_pool(name="wcol", bufs=1))
        w_col = wcol_pool.tile([128, c.TC], F32)
        wsT = ctx.enter_context(tc.tile_pool(name="wsT", bufs=1)).tile(
            [128, c.KC, c.Bc], BF16
        )
        hhT = ctx.enter_context(tc.tile_pool(name="hhT", bufs=1)).tile(
            [128, c.KC, c.Bc], BF16
        )
        hT = ctx.enter_context(tc.tile_pool(name="hT", bufs=1)).tile(
            [128, c.KC, c.B], BF16
        )

        if phase <= -1:
            _dbg_out(nc, tc, c, out, b1_sb)
            return
        # ---------------- Phase 1: router (fp32) + top-k + gather ----------
        # token-chunk-major layout so each (j, nt) gather dst is contiguous
        sel_es = contextlib.ExitStack()
        selT = sel_es.enter_context(tc.tile_pool(name="selT", bufs=1)).tile(
            [128, c.NG, c.K * c.DC, c.GC], BF16
        )
        # first expert-fc1 weight m-block, preloaded as two half-K tiles
        # during the sum-weights phase (the full-block pool opens at fc1).
        w1v = w1e.rearrange(
            "(kh kc p) (mg m) -> mg kh p kc m", p=128, kh=2, m=512
        )
        w1a_es = contextlib.ExitStack()
        w1a_pool = w1a_es.enter_context(tc.tile_pool(name="w1a", bufs=1))
        w1a = [w1a_pool.tile([128, c.KC // 2, 512], BF16, name=f"w1a{kh}")
               for kh in range(2)]

        # ---------------- Phase 2 pools (sum-weights fc1, feature-major) ---
        # swhT[f, b] = gelu(sum_k swW1[k, e*D+f] * xf[b, k]): lhsT = sw1s
        # chunk [128k, 128m], rhs = xfT chunk [128k, 512 tokens].
        swhT_es = contextlib.ExitStack()
        swhT = swhT_es.enter_context(tc.tile_pool(name="swhT", bufs=1)).tile(
            [128, c.DC, c.B], BF16
        )
        KB = 8                     # kc chunks per streamed block
        NKB = c.NDC // KB
        sw1v = sw1s.rearrange("(kb k p) m -> kb p k m", p=128, kb=NKB)
        xfv = xfT.rearrange("(kb k p) b -> kb p k b", p=128, kb=NKB)
        from concourse import library_config
        S = c.B // 16

        with tc.tile_pool(name="sww", bufs=2) as sww_pool, \
             tc.tile_pool(name="swx", bufs=2) as swx_pool, \
             tc.tile_pool(name="swf1_psum", bufs=1, space="PSUM") as swf1_psum:
            # first sw-fc1 block loads go ahead of the router's x load so the
            # PE can move straight from router matmuls into sw fc1.
            wblk00 = sww_pool.tile([128, KB, c.D], BF16, tag="sww", name="sww")
            xblk00 = swx_pool.tile([128, KB, 512], BF16, tag="swx", name="swx")

            with tc.tile_pool(name="rt", bufs=1) as rt_pool, \
                 tc.tile_pool(name="rt_psum", bufs=1, space="PSUM") as rt_psum:
                embT_sb = rt_pool.tile([128, c.DC, c.E], F32)
                nc.sync.dma_start(
                    embT_sb, embT.rearrange("(kc p) e -> p kc e", p=128)
                )
                xTv = xTe.rearrange("(kh kc p) b -> kh p kc b", p=128, kh=2)
                xTh0 = rt_pool.tile([128, 3, c.B], F32, tag="xTh", name="xTh")
                nc.sync.dma_start(xTh0, xTv[0])
                nc.sync.dma_start(wblk00, sw1v[0])
                nc.sync.dma_start(xblk00, xfv[0, :, :, 0:512])

                rt_ps = rt_psum.tile([128, c.TC, c.E], F32)
                for kh in range(2):
                    if kh == 0:
                        xTh = xTh0
                    else:
                        xTh = rt_pool.tile([128, 3, c.B], F32, tag="xTh",
                                           name="xTh")
                        nc.sync.dma_start(xTh, xTv[1])
                    for kc in range(3):
                        for t in range(c.TC):
                            nc.tensor.matmul(
                                rt_ps[:, t, :].opt(),
                                lhsT=xTh[:, kc, t * 128:(t + 1) * 128].opt(),
                                rhs=embT_sb[:, kh * 3 + kc, :].opt(),
                                start=(kh == 0 and kc == 0),
                                stop=(kh == 1 and kc == 2),
                            )
                idx_sb = rt_pool.tile([128, c.TC, c.K], I16, tag="idxsb",
                                      bufs=1)
                for t in range(c.TC):
                    lg = rt_pool.tile([128, c.E], F32, tag="lg", bufs=2)
                    nc.scalar.activation(lg, rt_ps[:, t, :].opt(), AF.Copy)
                    vals = rt_pool.tile([128, 8], F32, tag="vals", bufs=2)
                    nc.vector.max(out=vals, in_=lg)
                    idx8 = rt_pool.tile([128, 8], U16, tag="idx8", bufs=2)
                    nc.vector.max_index(out=idx8, in_max=vals, in_values=lg)
                    iota = rt_pool.tile([128, c.K], I16, tag="iota", bufs=2)
                    nc.gpsimd.iota(
                        iota, pattern=[[0, c.K]], base=t * 128 * c.E,
                        channel_multiplier=c.E,
                    )
                    nc.vector.tensor_add(
                        idx_sb[:, t, :].opt(), iota, idx8[:, 0:c.K].bitcast(I16)
                    )
                nc.sync.dma_start(
                    idx_dram.rearrange("(t p) j -> p t j", p=128), idx_sb
                )

                if phase <= 0:
                    _dbg_out(nc, tc, c, out, idx_sb[:, 0, :])
                    return

            for tb in range(2):
                psums = [
                    swf1_psum.tile([128, 512], F32, name=f"swps{mc}")
                    for mc in range(c.DC)
                ]
                for kb in range(NKB):
                    if tb == 0 and kb == 0:
                        wblk, xblk = wblk00, xblk00
                    else:
                        wblk = sww_pool.tile([128, KB, c.D], BF16, tag="sww",
                                             name="sww")
                        nc.sync.dma_start(wblk, sw1v[kb])
                        xblk = swx_pool.tile([128, KB, 512], BF16, tag="swx",
                                             name="swx")
                        nc.sync.dma_start(
                            xblk, xfv[kb, :, :, tb * 512:(tb + 1) * 512]
                        )
                    for mc in range(c.DC):
                        for k in range(KB):
                            nc.tensor.matmul(
                                psums[mc],
                                lhsT=wblk[:, k,
                                          mc * 128:(mc + 1) * 128].opt(),
                                rhs=xblk[:, k, :].opt(),
                                start=(kb == 0 and k == 0),
                                stop=(kb == NKB - 1 and k == KB - 1),
                            )
                for mc in range(c.DC):
                    nc.scalar.activation(
                        swhT[:, mc, tb * 512:(tb + 1) * 512].opt(),
                        psums[mc], AF.Gelu, bias=swb1_sb[:, mc:mc + 1],
                    )
                if tb == 0:
                    # ---- gather (indices -> selT) + consts, issued here so
                    # their DMAs interleave with the tb=1 stream ----------
                    nc.gpsimd.load_library(library_config.mlp)
                    with tc.tile_pool(name="gat", bufs=1) as gat_pool:
                        idxw_js = gat_pool.tile([128, S, c.K], I16)
                        iv = idx_dram.rearrange("(s p) j -> p s j", p=16)
                        for r in range(8):
                            nc.sync.dma_start(
                                idxw_js[16 * r:16 * r + 16, :, :], iv
                            )
                        idxw = gat_pool.tile([128, c.K, S], I16)
                        nc.vector.tensor_copy(
                            idxw, idxw_js[:, :, :].rearrange("p s j -> p j s")
                        )
                        load_consts()
                        for kh in range(2):
                            nc.sync.dma_start(w1a[kh], w1v[0, kh])
                        for g in range(c.NG):
                            for j in range(c.K):
                                nc.gpsimd.dma_gather(
                                    out_ap=selT[:, g, c.DC * j:c.DC * (j + 1), :],
                                    in_ap=xbf[:, :],
                                    idxs_ap=idxw[:, j, g * c.GC // 16:
                                                 (g + 1) * c.GC // 16],
                                    num_idxs=c.GC,
                                    num_idxs_reg=c.GC,
                                    elem_size=c.D,
                                    transpose=True,
                                )

        if phase <= 1:
            _dbg_out(nc, tc, c, out, selT[:, 0, 0, :])
            return

        with tc.tile_pool(name="swf2_psum", bufs=2, space="PSUM") as swf2_psum, \
             tc.tile_pool(name="swmisc", bufs=3) as swmisc:
            # fc2: partial logits, token-major [B, E]
            sw2_sb = swmisc.tile([128, c.DC, c.E], BF16, bufs=1)
            nc.sync.dma_start(sw2_sb, sw2s.rearrange("(kc p) e -> p kc e", p=128))
            wl_sb = swmisc.tile([128, c.TC, c.E], F32, bufs=1)
            for t in range(c.TC):
                ps = swf2_psum.tile([128, c.E], F32, name="swf2p")
                for kc in range(c.DC):
                    nc.tensor.matmul(
                        ps,
                        lhsT=swhT[:, kc, t * 128:(t + 1) * 128].opt(),
                        rhs=sw2_sb[:, kc, :].opt(),
                        start=(kc == 0),
                        stop=(kc == c.DC - 1),
                    )
                nc.scalar.activation(wl_sb[:, t, :].opt(), ps, AF.Copy)
            nc.sync.dma_start(
                wl_in.rearrange("(t p) e -> p t e", p=128), wl_sb
            )

        # AllReduce the sum-weights partial logits (tiny, overlaps fc1)
        nc.gpsimd.collective_compute(
            "AllReduce", ALU.add, replica_groups=rg,
            ins=[wl_in.opt()], outs=[wl_out.opt()],
        )
        swhT_es.close()

        if phase <= 2:
            _dbg_out(nc, tc, c, out, hT[:, 0, :] if False else selT[:, 0, 0, :])
            return
        # ---------------- Phase 3: expert fc1 ------------------------------
        KHC = c.KC // 2
        with tc.tile_pool(name="w1blk", bufs=2) as w1_pool, \
             tc.tile_pool(name="f1_psum", bufs=4, space="PSUM") as f1_psum:
            for mg in range(c.MG):
                if mg == 0:
                    def lhs_at(kc, mc):
                        return w1a[kc // KHC][:, kc % KHC,
                                              mc * 128:(mc + 1) * 128]
                else:
                    blk = w1_pool.tile([128, c.KC, 512], BF16, tag="w1b")
                    nc.sync.dma_start(
                        blk,
                        w1e[:, mg * 512:(mg + 1) * 512].rearrange(
                            "(kc p) m -> p kc m", p=128
                        ),
                    )

                    def lhs_at(kc, mc, blk=blk):
                        return blk[:, kc, mc * 128:(mc + 1) * 128]
                for mc in range(4):
                    m = mg * 4 + mc
                    for g in range(c.NG):
                        ps = f1_psum.tile([128, c.GC], F32, name="f1ps")
                        for kc in range(c.KC):
                            nc.tensor.matmul(
                                ps,
                                lhsT=lhs_at(kc, mc).opt(),
                                rhs=selT[:, g, kc, :].opt(),
                                start=(kc == 0),
                                stop=(kc == c.KC - 1),
                            )
                        nc.scalar.activation(
                            hT[:, m, g * c.GC:(g + 1) * c.GC].opt(),
                            ps, AF.Gelu, bias=b1_sb[:, m:m + 1],
                        )
        w1a_es.close()
        sel_es.close()

        if phase <= 3:
            _dbg_out(nc, tc, c, out, hT[:, 0, :])
            return
        # ---------------- softmax of routing weights (after AllReduce) -----
        # w_col[t] = softmax(wl + swb2)[:, e]  as per-partition scalars
        with tc.tile_pool(name="smx", bufs=2) as smx:
            wlf_all = smx.tile([128, c.TC, c.E], F32, bufs=1)
            nc.sync.dma_start(
                wlf_all, wl_out.rearrange("(t p) e -> p t e", p=128)
            )
            for t in range(c.TC):
                wlf = smx.tile([128, c.E], F32, tag="wlf")
                nc.vector.tensor_add(wlf, wlf_all[:, t, :].opt(), swb2_sb)
                mx = smx.tile([128, 1], F32, tag="mx")
                nc.vector.reduce_max(out=mx, in_=wlf, axis=mybir.AxisListType.X)
                nmx = smx.tile([128, 1], F32, tag="nmx")
                nc.vector.tensor_scalar_mul(nmx, mx, -1.0)
                ex = smx.tile([128, c.E], F32, tag="ex")
                sm = smx.tile([128, 1], F32, tag="sm")
                nc.scalar.activation(ex, wlf, AF.Exp, bias=nmx, accum_out=sm)
                rs = smx.tile([128, 1], F32, tag="rs")
                nc.vector.reciprocal(rs, sm)
                # pick this expert's column via onehot + row-sum, then scale
                sel = smx.tile([128, c.E], F32, tag="sel")
                nc.vector.tensor_mul(sel, ex, oh_sb)
                num = smx.tile([128, 1], F32, tag="num")
                nc.vector.reduce_sum(out=num, in_=sel, axis=mybir.AxisListType.X)
                nc.vector.tensor_tensor(
                    out=w_col[:, t:t + 1], in0=num, in1=rs, op=ALU.mult
                )

        # ---------------- Phase 4: expert fc2 (token-major) + scale -------
        # Token chunks split 768/256 so the final ReduceScatter is small and
        # its latency hides under the part-a head compute.
        NCOL = c.KD // 512
        TA = 6                  # token chunks in part a (768 tokens)
        TB = c.TC - TA          # part b (256 tokens)
        BA = TA * 128
        RA, RB = BA // NCORES, (c.B - BA) // NCORES   # 96 / 32 rows per core
        # w2 streamed as half-K blocks [128, 12, 512] so SBUF fits alongside
        # the chw1 (head fc1) prefetch pool.
        w2v = w2e.rearrange(
            "(kh kc p) (ncol m) -> ncol kh p kc m", p=128, m=512, kh=2
        )
        chv = chw1.rearrange("(kc p) (mg m) -> mg p kc m", p=128, m=512)
        pva = p_dram.rearrange(
            "(q ti p) (ncol m) -> q ncol p ti m", p=128, q=c.TC // 2, m=512
        )
        ch1_pool = ctx.enter_context(tc.tile_pool(name="ch1blk", bufs=3))
        ch1t = [ch1_pool.tile([128, c.KC, 512], BF16, tag="c1b", name="c1b")
                for mg in range(c.MG)]
        KH = c.KC // 2
        with tc.tile_pool(name="w2p", bufs=3) as w2_pool, \
             tc.tile_pool(name="f2_psum", bufs=2, space="PSUM") as f2_psum, \
             tc.tile_pool(name="pout", bufs=3) as p_pool:
            b2_sb = p_pool.tile([128, c.KD], F32, bufs=1)
            it = 0
            # ncol-outer within each token part: each w2 column block is
            # loaded once per part (2x total) instead of once per q group.
            for part, qr in ((0, range(0, TA // 2)), (1, range(TA // 2, 4))):
                for ncol in range(NCOL):
                    blks = []
                    for kh in range(2):
                        b_ = w2_pool.tile([128, KH, 512], BF16, tag="w2b",
                                          name="w2b")
                        nc.sync.dma_start(b_, w2v[ncol, kh])
                        blks.append(b_)
                    if it == 0:
                        nc.sync.dma_start(b2_sb, b2e[:, :])
                    if it < 3:              # prefetch first head fc1 blocks
                        nc.sync.dma_start(ch1t[it], chv[it])
                    it += 1
                    for q in qr:
                        psums = [
                            f2_psum.tile([128, 512], F32, name=f"f2ps{ti}")
                            for ti in range(2)
                        ]
                        pbig = p_pool.tile([128, 2, 512], BF16, tag="pbig")
                        for kc in range(c.KC):
                            for ti in range(2):
                                t = q * 2 + ti
                                nc.tensor.matmul(
                                    psums[ti],
                                    lhsT=hT[:, kc,
                                            t * 128:(t + 1) * 128].opt(),
                                    rhs=blks[kc // KH][:, kc % KH, :].opt(),
                                    start=(kc == 0),
                                    stop=(kc == c.KC - 1),
                                )
                        for ti in range(2):
                            t = q * 2 + ti
                            er = p_pool.tile([128, 512], F32, tag="er")
                            nc.vector.tensor_add(
                                er, psums[ti],
                                b2_sb[:, ncol * 512:(ncol + 1) * 512],
                            )
                            nc.vector.tensor_scalar_mul(
                                pbig[:, ti, :].opt(), er, w_col[:, t:t + 1]
                            )
                        nc.sync.dma_start(pva[q, ncol], pbig)
                if part == 0:
                    nc.gpsimd.collective_compute(
                        "ReduceScatter", ALU.add, replica_groups=rg,
                        ins=[p_dram[0:BA, :].opt()], outs=[ws_dram.opt()],
                    )
            nc.gpsimd.collective_compute(
                "ReduceScatter", ALU.add, replica_groups=rg,
                ins=[p_dram[BA:c.B, :].opt()], outs=[ws_dram2.opt()],
            )

        if phase <= 4:
            _dbg_out(nc, tc, c, out, w_col)
            return

        # ---------------- Phase 6: head on this core's Bc tokens -----------
        ch2_es = contextlib.ExitStack()
        ch2_pool = ch2_es.enter_context(tc.tile_pool(name="ch2", bufs=1))
        with tc.tile_pool(name="wst", bufs=2) as wst_pool, \
             tc.tile_pool(name="wst_psum", bufs=2, space="PSUM") as wst_psum, \
             tc.tile_pool(name="h1_psum", bufs=2, space="PSUM") as h1_psum:
            # part a (RA tokens, RS1 done early) runs while the small RS2 is
            # in flight; strict (a_i, b_i) pairs keep each chw1 block's two
            # uses adjacent so the 3-buffer stream window suffices.  DMA
            # order: wsa first (gates the transposes), then the chw2
            # prefetch, then wsb (waits on RS2 anyway).
            wsa = wst_pool.tile([RA, c.KD], BF16, bufs=1)
            nc.sync.dma_start(wsa, ws_dram[:, :])
            ch2t = []
            col0 = 0
            for i, ncols in enumerate(c.NCT):
                blk2 = ch2_pool.tile([128, c.KC, ncols], BF16, name=f"c2b{i}")
                nc.sync.dma_start(
                    blk2,
                    chw2[:, col0:col0 + ncols].rearrange(
                        "(kc p) m -> p kc m", p=128
                    ),
                )
                ch2t.append(blk2)
                col0 += ncols
            wsb = wst_pool.tile([RB, c.KD], BF16, bufs=1)
            nc.sync.dma_start(wsb, ws_dram2[:, :])

            def trans_part(src, r0, rn):
                for kc in range(c.KC):
                    tp = wst_psum.tile([128, rn], BF16, name="wstp")
                    nc.tensor.transpose(
                        tp, src[:, kc * 128:(kc + 1) * 128],
                        ident[0:rn, 0:rn],
                    )
                    nc.scalar.activation(
                        wsT[:, kc, r0:r0 + rn].opt(), tp, AF.Copy
                    )

            def fc1_part(mg, r0, rn):
                for mc in range(4):
                    m = mg * 4 + mc
                    ps = h1_psum.tile([128, rn], F32, name="h1ps")
                    for kc in range(c.KC):
                        nc.tensor.matmul(
                            ps,
                            lhsT=ch1t[mg][:, kc,
                                          mc * 128:(mc + 1) * 128].opt(),
                            rhs=wsT[:, kc, r0:r0 + rn].opt(),
                            start=(kc == 0),
                            stop=(kc == c.KC - 1),
                        )
                    nc.scalar.activation(
                        hhT[:, m, r0:r0 + rn].opt(), ps, AF.Gelu,
                        bias=chb1_sb[:, m:m + 1],
                    )

            trans_part(wsa, 0, RA)
            fc1_part(0, 0, RA)
            trans_part(wsb, RA, RB)
            for mg in range(c.MG):
                if mg + 3 < c.MG:
                    nc.sync.dma_start(ch1t[mg + 3], chv[mg + 3])
                if mg > 0:
                    fc1_part(mg, 0, RA)
                fc1_part(mg, RA, RB)
        with tc.tile_pool(name="h2_psum", bufs=2, space="PSUM") as h2_psum:
            col0 = 0
            for i, ncols in enumerate(c.NCT):
                ps = h2_psum.tile([c.Bc, ncols], F32, name="h2ps")
                for kc in range(c.KC):
                    nc.tensor.matmul(
                        ps,
                        lhsT=hhT[:, kc, :].opt(),
                        rhs=ch2t[i][:, kc, :].opt(),
                        start=(kc == 0),
                        stop=(kc == c.KC - 1),
                    )
                nc.vector.tensor_add(
                    out_sb[:, col0:col0 + ncols], ps,
                    chb2_sb[0:c.Bc, col0:col0 + ncols],
                )
                col0 += ncols
            nc.sync.dma_start(out[:, :], out_sb)
        ch2_es.close()


# ======================= host side =======================================

def prep_in_maps(inputs, cfg: Cfg):
    c = cfg
    bf = ml_dtypes.bfloat16
    x = np.asarray(inputs["x"], np.float32)
    emb = np.asarray(inputs["emb"], np.float32)
    W1 = np.asarray(inputs["W1"], np.float32)
    b1 = np.asarray(inputs["b1"], np.float32)
    W2 = np.asarray(inputs["W2"], np.float32)
    b2 = np.asarray(inputs["b2"], np.float32)
    swW1 = np.asarray(inputs["swW1"], np.float32)
    swb1 = np.asarray(inputs["swb1"], np.float32)
    swW2 = np.asarray(inputs["swW2"], np.float32)
    swb2 = np.asarray(inputs["swb2"], np.float32)
    chW1 = np.asarray(inputs["chW1"], np.float32)
    chb1 = np.asarray(inputs["chb1"], np.float32)
    chW2 = np.asarray(inputs["chW2"], np.float32)
    chb2 = np.asarray(inputs["chb2"], np.float32)

    embT = np.ascontiguousarray(emb.T)
    xbf = np.ascontiguousarray(x.reshape(c.B * c.E, c.D)).astype(bf)
    xfT = np.ascontiguousarray(x.reshape(c.B, c.ND).T).astype(bf)
    chw1_b = chW1.astype(bf)
    chw2_b = chW2.astype(bf)
    swb2_r = np.ascontiguousarray(np.broadcast_to(swb2.reshape(1, c.E), (128, c.E)))
    chb2_r = np.ascontiguousarray(np.broadcast_to(chb2.reshape(1, c.NCLS), (128, c.NCLS)))

    in_maps = []
    for e in range(NCORES):
        oh = np.zeros((128, c.E), np.float32)
        oh[:, e] = 1.0
        m = {
            "xTe": np.ascontiguousarray(x[:, e, :].T),
            "embT": embT,
            "xbf": xbf,
            "xfT": xfT,
            "sw1s": np.ascontiguousarray(swW1[:, e * c.D:(e + 1) * c.D]).astype(bf),
            "swb1c": np.ascontiguousarray(
                swb1[e * c.D:(e + 1) * c.D].reshape(c.DC, 128).T
            ),
            "sw2s": np.ascontiguousarray(swW2[e * c.D:(e + 1) * c.D, :]).astype(bf),
            "swb2": swb2_r,
            "w1e": W1[e].astype(bf),
            "b1e": b1[e],
            "w2e": W2[e].astype(bf),
            "b2e": np.ascontiguousarray(np.broadcast_to(b2[e].reshape(1, c.KD), (128, c.KD))),
            "chw1": chw1_b,
            "chb1": chb1,
            "chw2": chw2_b,
            "chb2": chb2_r,
            "onehot": oh,
        }
        in_maps.append(m)
    return in_maps


_CACHE = {}


def kernel(**inputs) -> np.ndarray:
    cfg = Cfg()
    key = "nc" + os.environ.get("KPHASE", "9")
    if key not in _CACHE:
        _CACHE[key] = build_nc(cfg)
    nc = _CACHE[key]
    in_maps = prep_in_maps(inputs, cfg)
    res = run_bass_kernel_spmd(
        nc, in_maps, core_ids=list(range(NCORES)),
        trace=bool(int(os.environ.get("KBENCH_TRACE", "0"))),
    )
    _CACHE["last_results"] = res
    ra, rb = cfg.B * 3 // 4 // NCORES, cfg.B // 4 // NCORES
    ba = cfg.B * 3 // 4
    outp = np.empty((cfg.B, cfg.NCLS), np.float32)
    for cix in range(NCORES):
        o = res.results[cix]["out"]
        outp[cix * ra:(cix + 1) * ra] = o[0:ra]
        outp[ba + cix * rb:ba + (cix + 1) * rb] = o[ra:]
    return outp

